# revision 23
# baseline (speedup 1.0000x reference)
"""Trainium2 Bass kernel for a DARTS RNN cell (T=256 steps, B=256, nhid=256).

Strategy
--------
Data-parallel over batch: 8 NeuronCores x 32 batch elements each; the tiny
weights (W0 [512,512], Ws [8,256,512]) are replicated. The T=256 recurrence is
sequential, computed fully on-chip.

Per-core layout is feature-major ("transposed"): every state tensor s^T lives
in one SBUF tile [128 partitions, 64] = (feature f%128 on partitions,
32*(f//128) + b on the free dim). Matmuls then run with the weight chunk
[128,128] (bf16, FWL fast weight load) stationary and the state chunk [128,32]
moving, producing feature-major PSUM directly -- no on-chip transposes at all.
The host pre-transposes/casts x, pre-chunks the weights into the exact SBUF
layout, and re-transposes the feature-major output.

Numerics: matmul operands bf16, accumulation + state updates + activations
fp32. (Measured vs the fp32 reference: rel l2 err ~6e-4, flat over t.)
The identity-activation step folds (W - I) into the weights host-side so the
"h - s" subtraction comes straight out of PSUM.
"""

import numpy as np
import ml_dtypes
from contextlib import ExitStack

import concourse.bass as bass
import concourse.tile as tile
from concourse import bacc, mybir
from concourse.bass_utils import run_bass_kernel_spmd

BF16 = ml_dtypes.bfloat16

GENOTYPE_RNN = [("sigmoid", 0), ("relu", 1), ("relu", 1), ("identity", 1),
                ("tanh", 2), ("sigmoid", 5), ("tanh", 3), ("relu", 5)]
T, B, NINP, NHID = 256, 256, 256, 256
N_CORES = 8
BL = B // N_CORES          # 32 batch elements per core
N_CHUNKS = 16 + 8 * 8      # W0 (4k x 4m) + 8 genotype (2k x 4m)
PREDS_USED = {0, 1, 2, 3, 5}   # states needed (bf16) as matmul moving operands

_ACT_FN = {"sigmoid": "Sigmoid", "tanh": "Tanh"}


def _chunk_index(step, k, m):
    """Column-chunk index of weight block (step, k, m) in the packed w_sb."""
    base = 0 if step == 0 else 16 + 8 * (step - 1)
    return base + k * 4 + m


def _pack_weights(W0, Ws):
    """Pack W0 / Ws (identity-folded for the identity step) into the SBUF
    layout [128, N_CHUNKS*128] bf16, chunk j at columns [128j, 128j+128)."""
    Wsf = np.array(Ws, dtype=np.float32, copy=True)
    for i, (name, _pred) in enumerate(GENOTYPE_RNN):
        if name == "identity":
            Wsf[i][:, NHID:] -= np.eye(NHID, dtype=np.float32)
    # The recurrence feeds h_prev into the next step as 8*h_new (the raw mean
    # tree sum, skipping a x0.125 op on the critical path); compensate by
    # scaling W0's h rows. Exact in fp: 0.125 is a pure exponent shift.
    W0f = np.array(W0, dtype=np.float32, copy=True)
    W0f[NHID:, :] *= 0.125
    w = np.zeros((128, N_CHUNKS * 128), dtype=BF16)
    for k in range(4):
        for m in range(4):
            j = _chunk_index(0, k, m)
            w[:, 128 * j:128 * (j + 1)] = W0f[128 * k:128 * (k + 1),
                                              128 * m:128 * (m + 1)].astype(BF16)
    for i in range(8):
        for k in range(2):
            for m in range(4):
                j = _chunk_index(i + 1, k, m)
                w[:, 128 * j:128 * (j + 1)] = Wsf[i][128 * k:128 * (k + 1),
                                                     128 * m:128 * (m + 1)].astype(BF16)
    return w


def _to_fm(a):
    """[T?, b, f] batch-major -> feature-major [T?, 128, 2, b] device layout."""
    a = np.asarray(a, dtype=np.float32)
    if a.ndim == 2:                      # [b, f]
        b, f = a.shape
        return a.T.reshape(2, 128, b).transpose(1, 0, 2)
    t, b, f = a.shape                    # [T, b, f]
    return a.transpose(0, 2, 1).reshape(t, 2, 128, b).transpose(0, 2, 1, 3)


def _from_fm(a):
    """[T, 128, 2, b] feature-major device layout -> [T, b, f]."""
    t = a.shape[0]
    return a.transpose(0, 2, 1, 3).reshape(t, NHID, -1).transpose(0, 2, 1)


def _build_program(n_t=T, passes=1):
    """Build + compile the per-core Bass/Tile program (identical on all cores).

    passes>1 repeats the whole recurrence (same inputs/outputs) -- only used
    to measure device-execution time as wall(passes=2) - wall(passes=1)."""
    f32 = mybir.dt.float32
    bf16 = mybir.dt.bfloat16
    AF = mybir.ActivationFunctionType
    ALU = mybir.AluOpType

    nc = bacc.Bacc("TRN2", target_bir_lowering=False, debug=False,
                   enable_asserts=False, enable_partition_id=False)

    x_d = nc.dram_tensor("x_fm", [n_t, 128, 2, BL], bf16, kind="ExternalInput").ap()
    w_d = nc.dram_tensor("w_sb", [128, N_CHUNKS * 128], bf16, kind="ExternalInput").ap()
    h0f_d = nc.dram_tensor("h0_f32", [128, 2, BL], f32, kind="ExternalInput").ap()
    h0b_d = nc.dram_tensor("h0_bf", [128, 2, BL], bf16, kind="ExternalInput").ap()
    out_d = nc.dram_tensor("out_fm", [n_t, 128, 2, BL], f32, kind="ExternalOutput").ap()

    FD = 2 * BL    # 64: free dim of one state tile

    # genotype emission order: dependency levels in order, chain step first
    # within a level. The engine sequencers execute in order (head-of-line
    # blocking), so a step must not be emitted before earlier-ready ones.
    # s7 (dep s3) goes before s6/s8 (dep s5) since s3 is ready earlier.
    STEP_ORDER = [0, 1, 2, 3, 4, 6, 5, 7]

    with tile.TileContext(nc) as tc, ExitStack() as ctx:
        wpool = ctx.enter_context(tc.tile_pool(name="w", bufs=1))
        xpool = ctx.enter_context(tc.tile_pool(name="x", bufs=8))
        spool = ctx.enter_context(tc.tile_pool(name="s", bufs=3))
        bfpool = ctx.enter_context(tc.tile_pool(name="sbf", bufs=3))
        tpool = ctx.enter_context(tc.tile_pool(name="tmp", bufs=4))
        opool = ctx.enter_context(tc.tile_pool(name="out", bufs=4))
        pspool = ctx.enter_context(tc.tile_pool(name="ps", bufs=3, space="PSUM"))

        w = wpool.tile([128, N_CHUNKS * 128], bf16, tag="w")
        nc.sync.dma_start(w, w_d)

        def wap(step, k, m):
            j = _chunk_index(step, k, m)
            return w[:, 128 * j:128 * (j + 1)]

        hprev_f = opool.tile([128, FD], f32, tag="hf")
        hprev_b = bfpool.tile([128, FD], bf16, tag="hb")
        nc.sync.dma_start(hprev_f.rearrange("p (c b) -> p c b", c=2), h0f_d)
        nc.sync.dma_start(hprev_b.rearrange("p (c b) -> p c b", c=2), h0b_d)

        for t in [t for _p in range(passes) for t in range(n_t)]:
            xt = xpool.tile([128, FD], bf16, tag="x")
            nc.sync.dma_start(xt.rearrange("p (c b) -> p c b", c=2), x_d[t])

            states_f = [None] * 9
            states_b = {}

            def mm_group(step, rhs_of_k, nk, psc, psh, h_first=True,
                         k_order=None):
                """Emit the matmul group. h-half (m=2,3) first when the chain
                goes through the h activation, c-half first when it goes
                through sigma (relu/identity steps). k_order lets step0 put
                the x-dependent contractions before the h_prev-dependent ones
                (PE executes in order; early ones run during the previous t)."""
                banks = ((psh, (2, 3)), (psc, (0, 1))) if h_first else \
                        ((psc, (0, 1)), (psh, (2, 3)))
                ks = k_order or range(nk)
                for bank, ms in banks:
                    first = True
                    last = (ms[1], ks[-1] if k_order else nk - 1)
                    for k in ks:
                        for m in ms:
                            nc.tensor.matmul(bank[:, BL * (m % 2):BL * (m % 2 + 1)],
                                             lhsT=wap(step, k, m), rhs=rhs_of_k(k),
                                             start=first,
                                             stop=((m, k) == last))
                            first = False

            def update(step_i, name, spf, spb_needed, psc, psh, fused=False):
                """activation + state increment; returns (e, s_f32|None,
                s_bf16|None). Chain order: act(h) -> d -> e [-> s_bf16] (DVE);
                sigma(c) runs in the gaps. The fp32 state (needed only when
                this state is some step's predecessor) lands off the critical
                path on GPSIMD; non-pred states are never materialized -- the
                mean uses the e increments directly."""
                if fused:   # all-sigmoid step: one ACT over the whole bank
                    ch = tpool.tile([128, 2 * FD], f32, tag="ch")
                    nc.scalar.activation(ch, psc, AF.Sigmoid)
                    c, h = ch[:, :FD], ch[:, FD:]
                    d = tpool.tile([128, FD], f32, tag="d")
                    nc.vector.tensor_sub(d, h, spf)
                elif name in _ACT_FN:
                    c = tpool.tile([128, FD], f32, tag="c")
                    h = tpool.tile([128, FD], f32, tag="h")
                    nc.scalar.activation(h, psh, getattr(AF, _ACT_FN[name]))
                    nc.scalar.activation(c, psc, AF.Sigmoid)
                    d = tpool.tile([128, FD], f32, tag="d")
                    nc.vector.tensor_sub(d, h, spf)
                elif name == "relu":
                    c = tpool.tile([128, FD], f32, tag="c")
                    nc.scalar.activation(c, psc, AF.Sigmoid)
                    d = tpool.tile([128, FD], f32, tag="d")
                    nc.vector.scalar_tensor_tensor(d, psh, 0.0, spf,
                                                   op0=ALU.max, op1=ALU.subtract)
                else:  # identity: (W - I) folded host-side, psh already holds h-s
                    c = tpool.tile([128, FD], f32, tag="c")
                    nc.scalar.activation(c, psc, AF.Sigmoid)
                    d = psh
                e = tpool.tile([128, FD], f32, tag=f"e{step_i}")
                nc.vector.tensor_mul(e, c, d)
                if not spb_needed:
                    return e, None, None
                sb = bfpool.tile([128, FD], bf16, tag=f"s{step_i}b")
                nc.vector.tensor_add(sb, spf, e)           # chain: bf16 operand
                sf = spool.tile([128, FD], f32, tag=f"s{step_i}")
                nc.gpsimd.tensor_add(sf, spf, e)           # fp32 copy off-chain
                return e, sf, sb

            # ---- init step: ch0 = [x, 8*h_prev] @ [W0a; W0b/8] (feature-major)
            psc = pspool.tile([128, FD], f32, tag="psc")
            psh = pspool.tile([128, FD], f32, tag="psh")
            mm_group(0, lambda k: (xt if k < 2 else hprev_b)[:, BL * (k % 2):BL * (k % 2 + 1)],
                     4, psc, psh, h_first=True, k_order=(0, 1, 2, 3))
            _e0, states_f[0], states_b[0] = update(0, "tanh", hprev_f, True, psc, psh)

            # ---- genotype steps, dependency-level emission; keep increments
            es = [None] * 9
            for i in STEP_ORDER:
                name, pred = GENOTYPE_RNN[i]
                if name == "sigmoid":   # one bank + one fused sigma over c|h
                    psf = pspool.tile([128, 2 * FD], f32, tag="psf", bufs=2)
                    first = True
                    for k in range(2):
                        for m in range(4):
                            nc.tensor.matmul(psf[:, BL * m:BL * (m + 1)],
                                             lhsT=wap(i + 1, k, m),
                                             rhs=states_b[pred][:, BL * k:BL * (k + 1)],
                                             start=first, stop=(k == 1 and m == 3))
                            first = False
                    e, sf, sb = update(i + 1, name, states_f[pred],
                                       (i + 1) in PREDS_USED, psf, None,
                                       fused=True)
                else:
                    psc = pspool.tile([128, FD], f32, tag="psc")
                    psh = pspool.tile([128, FD], f32, tag="psh")
                    mm_group(i + 1, lambda k: states_b[pred][:, BL * k:BL * (k + 1)],
                             2, psc, psh, h_first=(name in _ACT_FN))
                    e, sf, sb = update(i + 1, name, states_f[pred],
                                       (i + 1) in PREDS_USED, psc, psh)
                es[i + 1] = e
                if sf is not None:
                    states_f[i + 1] = sf
                    states_b[i + 1] = sb

            # ---- 8*h_new = sum(s1..s8) = s0 + 3*s1 + s2 + s3 + 2*s5 + sum(e)
            # (preds = [0,1,1,1,2,5,3,5]). All state-weighted partials and
            # early e-sums run off-chain on GPSIMD; the chain tail after the
            # last increment e8 is a single DVE add producing hb (bf16) for
            # the next matmul (W0b pre-scaled x1/8). The fp32 output is the
            # raw sum, rescaled x0.125 on the host.
            # w1 = 3*s1 + s0, w4 = 2*s5 + w3, decomposed into plain adds so
            # everything runs on GPSIMD (exact same fp32 values: 3x = (x+x)+x
            # rounds identically to fp32 3*x? -- no: keep the model in sync
            # instead, it mirrors this exact association).
            w1a = tpool.tile([128, FD], f32, tag="w1a")
            nc.gpsimd.tensor_add(w1a, states_f[1], states_f[1])
            w1b = tpool.tile([128, FD], f32, tag="w1b")
            nc.gpsimd.tensor_add(w1b, states_f[1], states_f[0])
            w1 = tpool.tile([128, FD], f32, tag="w1")
            nc.gpsimd.tensor_add(w1, w1a, w1b)
            w2 = tpool.tile([128, FD], f32, tag="w2")
            nc.gpsimd.tensor_add(w2, w1, states_f[2])
            w3 = tpool.tile([128, FD], f32, tag="w3")
            nc.gpsimd.tensor_add(w3, w2, states_f[3])
            w4a = tpool.tile([128, FD], f32, tag="w4a")
            nc.gpsimd.tensor_add(w4a, states_f[5], states_f[5])
            w4 = tpool.tile([128, FD], f32, tag="w4")
            nc.gpsimd.tensor_add(w4, w4a, w3)
            e12 = tpool.tile([128, FD], f32, tag="e12")
            nc.vector.tensor_add(e12, es[1], es[2])
            e34 = tpool.tile([128, FD], f32, tag="e34")
            nc.vector.tensor_add(e34, es[3], es[4])
            ea = tpool.tile([128, FD], f32, tag="ea")
            nc.gpsimd.tensor_add(ea, e12, e34)
            e57 = tpool.tile([128, FD], f32, tag="e57")
            nc.vector.tensor_add(e57, es[5], es[7])
            eb = tpool.tile([128, FD], f32, tag="eb")
            nc.gpsimd.tensor_add(eb, ea, e57)
            fpart = tpool.tile([128, FD], f32, tag="fp")
            nc.gpsimd.tensor_add(fpart, w4, eb)
            z = tpool.tile([128, FD], f32, tag="z")
            nc.gpsimd.tensor_add(z, fpart, es[6])
            hprev_b = bfpool.tile([128, FD], bf16, tag="hb")
            nc.vector.tensor_add(hprev_b, z, es[8])             # chain: next mm
            hsum = tpool.tile([128, FD], f32, tag="hs")
            nc.gpsimd.tensor_add(hsum, z, es[8])
            hprev_f = opool.tile([128, FD], f32, tag="hf")
            nc.gpsimd.tensor_scalar_mul(hprev_f, hsum, 0.125)

            nc.sync.dma_start(out_d[t], hsum.rearrange("p (c b) -> p c b", c=2))

    nc.compile()
    return nc


_PROGRAM_CACHE = {}


def get_program(n_t=T, passes=1):
    key = (n_t, passes)
    if key not in _PROGRAM_CACHE:
        _PROGRAM_CACHE[key] = _build_program(n_t, passes)
    return _PROGRAM_CACHE[key]


def make_in_maps(inputs_x, hidden, W0, Ws):
    """Host-side prep: shard batch, pack weights, transpose to device layout."""
    w_sb = _pack_weights(np.asarray(W0, np.float32), np.asarray(Ws, np.float32))
    x = np.asarray(inputs_x, np.float32)
    h0 = np.asarray(hidden, np.float32)[0]
    in_maps = []
    for core in range(N_CORES):
        sl = slice(core * BL, (core + 1) * BL)
        h0_fm = _to_fm(h0[sl])
        in_maps.append({
            "x_fm": _to_fm(x[:, sl, :]).astype(BF16),
            "w_sb": w_sb,
            "h0_f32": h0_fm,
            # the recurrence carries 8*h as the bf16 matmul operand
            "h0_bf": (8.0 * h0_fm).astype(BF16),
        })
    return in_maps


def run(inputs, hidden, W0, Ws, n_t=T, trace=False, passes=1, in_maps=None,
        **spmd_kwargs):
    nc = get_program(n_t, passes)
    if in_maps is None:
        in_maps = make_in_maps(inputs, hidden, W0, Ws)
    res = run_bass_kernel_spmd(nc, in_maps, core_ids=list(range(N_CORES)),
                               trace=trace, **spmd_kwargs)
    hid = np.concatenate([_from_fm(r["out_fm"]) for r in res.results], axis=1)
    hid = (hid * np.float32(0.125)).astype(np.float32)   # device emits 8*h
    return hid, res


def kernel(inputs, hidden, W0, Ws, rnn_mask=None, **_ignored):
    hiddens, _res = run(inputs, hidden, W0, Ws)
    return hiddens, hiddens[-1][None]


# revision 25
# speedup vs baseline: 8138.5326x; 8138.5326x over previous
"""Trainium2 Bass kernel for a DARTS RNN cell (T=256 steps, B=256, nhid=256).

Strategy
--------
Data-parallel over batch: 8 NeuronCores x 32 batch elements each; the tiny
weights (W0 [512,512], Ws [8,256,512]) are replicated. The T=256 recurrence is
sequential, computed fully on-chip.

Per-core layout is feature-major ("transposed"): every state tensor s^T lives
in one SBUF tile [128 partitions, 64] = (feature f%128 on partitions,
32*(f//128) + b on the free dim). Matmuls then run with the weight chunk
[128,128] (bf16, FWL fast weight load) stationary and the state chunk [128,32]
moving, producing feature-major PSUM directly -- no on-chip transposes at all.
The host pre-transposes/casts x, pre-chunks the weights into the exact SBUF
layout, and re-transposes the feature-major output.

Numerics: matmul operands bf16, accumulation + state updates + activations
fp32. (Measured vs the fp32 reference: rel l2 err ~6e-4, flat over t.)
The identity-activation step folds (W - I) into the weights host-side so the
"h - s" subtraction comes straight out of PSUM.
"""

import numpy as np
import ml_dtypes
from contextlib import ExitStack

import concourse.bass as bass
import concourse.tile as tile
from concourse import bacc, mybir
from concourse.bass_utils import run_bass_kernel_spmd

BF16 = ml_dtypes.bfloat16

GENOTYPE_RNN = [("sigmoid", 0), ("relu", 1), ("relu", 1), ("identity", 1),
                ("tanh", 2), ("sigmoid", 5), ("tanh", 3), ("relu", 5)]
T, B, NINP, NHID = 256, 256, 256, 256
N_CORES = 8
BL = B // N_CORES          # 32 batch elements per core
N_CHUNKS = 16 + 8 * 8      # W0 (4k x 4m) + 8 genotype (2k x 4m)
PREDS_USED = {0, 1, 2, 3, 5}   # states needed (bf16) as matmul moving operands

_ACT_FN = {"sigmoid": "Sigmoid", "tanh": "Tanh"}


def _chunk_index(step, k, m):
    """Column-chunk index of weight block (step, k, m) in the packed w_sb."""
    base = 0 if step == 0 else 16 + 8 * (step - 1)
    return base + k * 4 + m


def _pack_weights(W0, Ws):
    """Pack W0 / Ws (identity-folded for the identity step) into the SBUF
    layout [128, N_CHUNKS*128] bf16, chunk j at columns [128j, 128j+128)."""
    Wsf = np.array(Ws, dtype=np.float32, copy=True)
    for i, (name, _pred) in enumerate(GENOTYPE_RNN):
        if name == "identity":
            Wsf[i][:, NHID:] -= np.eye(NHID, dtype=np.float32)
    # The recurrence feeds h_prev into the next step as 8*h_new (the raw mean
    # tree sum, skipping a x0.125 op on the critical path); compensate by
    # scaling W0's h rows. Exact in fp: 0.125 is a pure exponent shift.
    W0f = np.array(W0, dtype=np.float32, copy=True)
    W0f[NHID:, :] *= 0.125
    w = np.zeros((128, N_CHUNKS * 128), dtype=BF16)
    for k in range(4):
        for m in range(4):
            j = _chunk_index(0, k, m)
            w[:, 128 * j:128 * (j + 1)] = W0f[128 * k:128 * (k + 1),
                                              128 * m:128 * (m + 1)].astype(BF16)
    for i in range(8):
        for k in range(2):
            for m in range(4):
                j = _chunk_index(i + 1, k, m)
                w[:, 128 * j:128 * (j + 1)] = Wsf[i][128 * k:128 * (k + 1),
                                                     128 * m:128 * (m + 1)].astype(BF16)
    return w


def _to_fm(a):
    """[T?, b, f] batch-major -> feature-major [T?, 128, 2, b] device layout."""
    a = np.asarray(a, dtype=np.float32)
    if a.ndim == 2:                      # [b, f]
        b, f = a.shape
        return a.T.reshape(2, 128, b).transpose(1, 0, 2)
    t, b, f = a.shape                    # [T, b, f]
    return a.transpose(0, 2, 1).reshape(t, 2, 128, b).transpose(0, 2, 1, 3)


def _from_fm(a):
    """[T, 128, 2, b] feature-major device layout -> [T, b, f]."""
    t = a.shape[0]
    return a.transpose(0, 2, 1, 3).reshape(t, NHID, -1).transpose(0, 2, 1)


def _build_program(n_t=T, passes=1):
    """Build + compile the per-core Bass/Tile program (identical on all cores).

    passes>1 repeats the whole recurrence (same inputs/outputs) -- only used
    to measure device-execution time as wall(passes=2) - wall(passes=1)."""
    f32 = mybir.dt.float32
    bf16 = mybir.dt.bfloat16
    AF = mybir.ActivationFunctionType
    ALU = mybir.AluOpType

    nc = bacc.Bacc("TRN2", target_bir_lowering=False, debug=False,
                   enable_asserts=False, enable_partition_id=False)

    x_d = nc.dram_tensor("x_fm", [n_t, 128, 2, BL], bf16, kind="ExternalInput").ap()
    w_d = nc.dram_tensor("w_sb", [128, N_CHUNKS * 128], bf16, kind="ExternalInput").ap()
    h0f_d = nc.dram_tensor("h0_f32", [128, 2, BL], f32, kind="ExternalInput").ap()
    h0b_d = nc.dram_tensor("h0_bf", [128, 2, BL], bf16, kind="ExternalInput").ap()
    out_d = nc.dram_tensor("out_fm", [n_t, 128, 2, BL], f32, kind="ExternalOutput").ap()

    FD = 2 * BL    # 64: free dim of one state tile

    # genotype emission order: dependency levels in order, chain step first
    # within a level. The engine sequencers execute in order (head-of-line
    # blocking), so a step must not be emitted before earlier-ready ones.
    # s7 (dep s3) goes before s6/s8 (dep s5) since s3 is ready earlier.
    STEP_ORDER = [0, 1, 2, 3, 4, 6, 5, 7]

    with tile.TileContext(nc) as tc, ExitStack() as ctx:
        wpool = ctx.enter_context(tc.tile_pool(name="w", bufs=1))
        xpool = ctx.enter_context(tc.tile_pool(name="x", bufs=8))
        spool = ctx.enter_context(tc.tile_pool(name="s", bufs=3))
        bfpool = ctx.enter_context(tc.tile_pool(name="sbf", bufs=3))
        tpool = ctx.enter_context(tc.tile_pool(name="tmp", bufs=4))
        opool = ctx.enter_context(tc.tile_pool(name="out", bufs=4))
        pspool = ctx.enter_context(tc.tile_pool(name="ps", bufs=3, space="PSUM"))

        w = wpool.tile([128, N_CHUNKS * 128], bf16, tag="w")
        nc.sync.dma_start(w, w_d)

        def wap(step, k, m):
            j = _chunk_index(step, k, m)
            return w[:, 128 * j:128 * (j + 1)]

        hprev_f = opool.tile([128, FD], f32, tag="hf")
        hprev_b = bfpool.tile([128, FD], bf16, tag="hb")
        nc.sync.dma_start(hprev_f.rearrange("p (c b) -> p c b", c=2), h0f_d)
        nc.sync.dma_start(hprev_b.rearrange("p (c b) -> p c b", c=2), h0b_d)

        for t in [t for _p in range(passes) for t in range(n_t)]:
            xt = xpool.tile([128, FD], bf16, tag="x")
            nc.sync.dma_start(xt.rearrange("p (c b) -> p c b", c=2), x_d[t])

            states_f = [None] * 9
            states_b = {}

            def mm_group(step, rhs_of_k, nk, psc, psh, h_first=True,
                         k_order=None):
                """Emit the matmul group. h-half (m=2,3) first when the chain
                goes through the h activation, c-half first when it goes
                through sigma (relu/identity steps). k_order lets step0 put
                the x-dependent contractions before the h_prev-dependent ones
                (PE executes in order; early ones run during the previous t)."""
                banks = ((psh, (2, 3)), (psc, (0, 1))) if h_first else \
                        ((psc, (0, 1)), (psh, (2, 3)))
                ks = k_order or range(nk)
                for bank, ms in banks:
                    first = True
                    last = (ms[1], ks[-1] if k_order else nk - 1)
                    for k in ks:
                        for m in ms:
                            nc.tensor.matmul(bank[:, BL * (m % 2):BL * (m % 2 + 1)],
                                             lhsT=wap(step, k, m), rhs=rhs_of_k(k),
                                             start=first,
                                             stop=((m, k) == last))
                            first = False

            def update(step_i, name, spf, spb_needed, psc, psh, fused=False):
                """activation + state increment; returns (e, s_f32|None,
                s_bf16|None). Chain order: act(h) -> d -> e [-> s_bf16] (DVE);
                sigma(c) runs in the gaps. The fp32 state (needed only when
                this state is some step's predecessor) lands off the critical
                path on GPSIMD; non-pred states are never materialized -- the
                mean uses the e increments directly."""
                if fused:   # all-sigmoid step: one ACT over the whole bank
                    ch = tpool.tile([128, 2 * FD], f32, tag="ch")
                    nc.scalar.activation(ch, psc, AF.Sigmoid)
                    c, h = ch[:, :FD], ch[:, FD:]
                    d = tpool.tile([128, FD], f32, tag="d")
                    nc.vector.tensor_sub(d, h, spf)
                elif name in _ACT_FN:
                    c = tpool.tile([128, FD], f32, tag="c")
                    h = tpool.tile([128, FD], f32, tag="h")
                    nc.scalar.activation(h, psh, getattr(AF, _ACT_FN[name]))
                    nc.scalar.activation(c, psc, AF.Sigmoid)
                    d = tpool.tile([128, FD], f32, tag="d")
                    nc.vector.tensor_sub(d, h, spf)
                elif name == "relu":
                    c = tpool.tile([128, FD], f32, tag="c")
                    nc.scalar.activation(c, psc, AF.Sigmoid)
                    d = tpool.tile([128, FD], f32, tag="d")
                    nc.vector.scalar_tensor_tensor(d, psh, 0.0, spf,
                                                   op0=ALU.max, op1=ALU.subtract)
                else:  # identity: (W - I) folded host-side, psh already holds h-s
                    c = tpool.tile([128, FD], f32, tag="c")
                    nc.scalar.activation(c, psc, AF.Sigmoid)
                    d = psh
                e = tpool.tile([128, FD], f32, tag=f"e{step_i}")
                nc.vector.tensor_mul(e, c, d)
                if not spb_needed:
                    return e, None, None
                sb = bfpool.tile([128, FD], bf16, tag=f"s{step_i}b")
                nc.vector.tensor_add(sb, spf, e)           # chain: bf16 operand
                sf = spool.tile([128, FD], f32, tag=f"s{step_i}")
                nc.gpsimd.tensor_add(sf, spf, e)           # fp32 copy off-chain
                return e, sf, sb

            # ---- init step: ch0 = [x, 8*h_prev] @ [W0a; W0b/8] (feature-major)
            psc = pspool.tile([128, FD], f32, tag="psc")
            psh = pspool.tile([128, FD], f32, tag="psh")
            mm_group(0, lambda k: (xt if k < 2 else hprev_b)[:, BL * (k % 2):BL * (k % 2 + 1)],
                     4, psc, psh, h_first=True, k_order=(0, 1, 2, 3))
            _e0, states_f[0], states_b[0] = update(0, "tanh", hprev_f, True, psc, psh)

            # ---- genotype steps, dependency-level emission; keep increments
            es = [None] * 9
            for i in STEP_ORDER:
                name, pred = GENOTYPE_RNN[i]
                psc = pspool.tile([128, FD], f32, tag="psc")
                psh = pspool.tile([128, FD], f32, tag="psh")
                mm_group(i + 1, lambda k: states_b[pred][:, BL * k:BL * (k + 1)],
                         2, psc, psh, h_first=(name in _ACT_FN))
                e, sf, sb = update(i + 1, name, states_f[pred],
                                   (i + 1) in PREDS_USED, psc, psh)
                es[i + 1] = e
                if sf is not None:
                    states_f[i + 1] = sf
                    states_b[i + 1] = sb

            # ---- 8*h_new = sum(s1..s8) = s0 + 3*s1 + s2 + s3 + 2*s5 + sum(e)
            # (preds = [0,1,1,1,2,5,3,5]). All state-weighted partials and
            # early e-sums run off-chain on GPSIMD; the chain tail after the
            # last increment e8 is a single DVE add producing hb (bf16) for
            # the next matmul (W0b pre-scaled x1/8). The fp32 output is the
            # raw sum, rescaled x0.125 on the host.
            w1 = tpool.tile([128, FD], f32, tag="w1")
            nc.vector.scalar_tensor_tensor(w1, states_f[1], 3.0, states_f[0],
                                           op0=ALU.mult, op1=ALU.add)
            w2 = tpool.tile([128, FD], f32, tag="w2")
            nc.gpsimd.tensor_add(w2, w1, states_f[2])
            w3 = tpool.tile([128, FD], f32, tag="w3")
            nc.gpsimd.tensor_add(w3, w2, states_f[3])
            w4 = tpool.tile([128, FD], f32, tag="w4")
            nc.vector.scalar_tensor_tensor(w4, states_f[5], 2.0, w3,
                                           op0=ALU.mult, op1=ALU.add)
            e12 = tpool.tile([128, FD], f32, tag="e12")
            nc.vector.tensor_add(e12, es[1], es[2])
            e34 = tpool.tile([128, FD], f32, tag="e34")
            nc.vector.tensor_add(e34, es[3], es[4])
            ea = tpool.tile([128, FD], f32, tag="ea")
            nc.gpsimd.tensor_add(ea, e12, e34)
            e57 = tpool.tile([128, FD], f32, tag="e57")
            nc.vector.tensor_add(e57, es[5], es[7])
            eb = tpool.tile([128, FD], f32, tag="eb")
            nc.gpsimd.tensor_add(eb, ea, e57)
            fpart = tpool.tile([128, FD], f32, tag="fp")
            nc.gpsimd.tensor_add(fpart, w4, eb)
            z = tpool.tile([128, FD], f32, tag="z")
            nc.gpsimd.tensor_add(z, fpart, es[6])
            hprev_b = bfpool.tile([128, FD], bf16, tag="hb")
            nc.vector.tensor_add(hprev_b, z, es[8])             # chain: next mm
            hsum = tpool.tile([128, FD], f32, tag="hs")
            nc.gpsimd.tensor_add(hsum, z, es[8])
            hprev_f = opool.tile([128, FD], f32, tag="hf")
            nc.gpsimd.tensor_scalar_mul(hprev_f, hsum, 0.125)

            nc.sync.dma_start(out_d[t], hsum.rearrange("p (c b) -> p c b", c=2))

    nc.compile()
    return nc


_PROGRAM_CACHE = {}


def get_program(n_t=T, passes=1):
    key = (n_t, passes)
    if key not in _PROGRAM_CACHE:
        _PROGRAM_CACHE[key] = _build_program(n_t, passes)
    return _PROGRAM_CACHE[key]


def make_in_maps(inputs_x, hidden, W0, Ws):
    """Host-side prep: shard batch, pack weights, transpose to device layout."""
    w_sb = _pack_weights(np.asarray(W0, np.float32), np.asarray(Ws, np.float32))
    x = np.asarray(inputs_x, np.float32)
    h0 = np.asarray(hidden, np.float32)[0]
    in_maps = []
    for core in range(N_CORES):
        sl = slice(core * BL, (core + 1) * BL)
        h0_fm = _to_fm(h0[sl])
        in_maps.append({
            "x_fm": _to_fm(x[:, sl, :]).astype(BF16),
            "w_sb": w_sb,
            "h0_f32": h0_fm,
            # the recurrence carries 8*h as the bf16 matmul operand
            "h0_bf": (8.0 * h0_fm).astype(BF16),
        })
    return in_maps


def run(inputs, hidden, W0, Ws, n_t=T, trace=False, passes=1, in_maps=None,
        **spmd_kwargs):
    nc = get_program(n_t, passes)
    if in_maps is None:
        in_maps = make_in_maps(inputs, hidden, W0, Ws)
    res = run_bass_kernel_spmd(nc, in_maps, core_ids=list(range(N_CORES)),
                               trace=trace, **spmd_kwargs)
    hid = np.concatenate([_from_fm(r["out_fm"]) for r in res.results], axis=1)
    hid = (hid * np.float32(0.125)).astype(np.float32)   # device emits 8*h
    return hid, res


def kernel(inputs, hidden, W0, Ws, rnn_mask=None, **_ignored):
    hiddens, _res = run(inputs, hidden, W0, Ws)
    return hiddens, hiddens[-1][None]
